# revision 1
# baseline (speedup 1.0000x reference)
"""Causal self-attention (B=4, T=2048, C=1024, NH=16) on 8 TRN2 NeuronCores.

Sharding: tensor-parallel over heads — 2 heads per core. Each core computes
its slice of qkv (transposed layout), full causal attention for its heads,
and a partial output projection; the host sums the 8 partials and adds b_proj.

Matmuls run in float32r (fp32 bits, reduced-precision PE mode, ~1.6e-4 rel
err) which streams at bf16 speed for free dims >= 256.

Layout notes:
 - qkv is computed transposed (qT/kT/vT: [dim, tok]) so scores can be formed
   as scoresT[k, q] = kT.T @ qT with d on partitions; softmax runs over the
   partition (k) axis using exp on ACT, a ones-column in the p@v matmul for
   the denominator, and a K=1 broadcast matmul for the reciprocal.
 - x is transposed on the host once (layout prep during sharding) so qkv
   needs no on-device transposes; v is re-transposed to natural layout on PE.
"""

import sys

import numpy as np

try:
    import concourse.bass as bass
except ImportError:  # grading container may not have it on sys.path
    sys.path.insert(0, "/opt/trn_rl_repo")
    import concourse.bass as bass

from contextlib import ExitStack

import concourse.mybir as mybir
import concourse.tile as tile
from concourse.bass_utils import run_bass_kernel_spmd


B, T, C, NH, HD = 4, 2048, 1024, 16, 64
N_CORES = 8
HPC = NH // N_CORES  # heads per core = 2
DPC = HPC * HD  # dims per core = 128
BT = B * T  # 8192
QCH = 512  # q-chunk (moving free dim)
KCH = 128  # k-chunk (contraction tile)
TCH = 512  # token chunk for qkv
F32 = mybir.dt.float32
F32R = mybir.dt.float32r
AF = mybir.ActivationFunctionType


def _r(ap):
    return ap.bitcast(F32R)


def _act_reciprocal(nc, out, in_):
    """Reciprocal on the scalar engine (~430ns for [1,512] vs ~3.3us for
    nc.vector.reciprocal's Newton chain). bass blocks AF.Reciprocal behind an
    accuracy warning; the spline is good to ~1e-5 rel which is far below this
    kernel's f32r noise floor, so emit the instruction directly."""
    eng = nc.scalar
    ins = [
        eng.lower_ap(in_),
        mybir.ImmediateValue(dtype=mybir.dt.float32, value=0.0),
        mybir.ImmediateValue(dtype=mybir.dt.float32, value=1.0),
        mybir.ImmediateValue(dtype=mybir.dt.float32, value=0.0),
    ]
    return eng.add_instruction(
        mybir.InstActivation(
            name=nc.get_next_instruction_name(),
            func=AF.Reciprocal,
            ins=ins,
            outs=[eng.lower_ap(out)],
        )
    )


def _split_multi_waits(nc):
    """Walrus in this container accepts only ONE sync wait per instruction.
    Hoist extra waits onto same-engine NoOps inserted just before."""
    n = 0
    for f in nc.m.functions:
        for b in f.blocks:
            insts = b.instructions
            if not any(
                i.sync_info is not None
                and i.sync_info.on_wait
                and len(i.sync_info.on_wait) > 1
                for i in insts
            ):
                continue
            new = []
            for ins in insts:
                si = ins.sync_info
                if si is not None and si.on_wait and len(si.on_wait) > 1:
                    waits = list(si.on_wait)
                    for w in waits[:-1]:
                        nop = mybir.InstNoOp(
                            name=f"{ins.name}-ws{n}", ins=[], outs=[]
                        )
                        nop.engine = ins.engine
                        nop.bass_nofuse = True
                        nop.sync_info = mybir.SyncInfo(on_wait=[w], on_update=[])
                        if ins.debug is not None:
                            nop.debug = ins.debug
                        new.append(nop)
                        n += 1
                    ins.sync_info = mybir.SyncInfo(
                        on_wait=[waits[-1]], on_update=list(si.on_update or [])
                    )
                new.append(ins)
            b.instructions = new
    return n


def build_kernel():
    nc = bass.Bass("TRN2", target_bir_lowering=False, debug=False, num_devices=N_CORES)
    xT_d = nc.dram_tensor("xT", [C, BT], F32R, kind="ExternalInput")
    wc_d = nc.dram_tensor("wc", [C, 3 * DPC], F32R, kind="ExternalInput")
    bc_d = nc.dram_tensor("bc", [3, DPC, 1], F32, kind="ExternalInput")
    wp_d = nc.dram_tensor("wp", [DPC, C], F32R, kind="ExternalInput")
    out_d = nc.dram_tensor("out", [BT, C], F32, kind="ExternalOutput")

    with tile.TileContext(nc) as tc, ExitStack() as ctx:
        consts = ctx.enter_context(tc.tile_pool(name="consts", bufs=1))
        xpool = ctx.enter_context(tc.tile_pool(name="x", bufs=16))
        qkvp = ctx.enter_context(tc.tile_pool(name="qkv", bufs=2))
        vexp = ctx.enter_context(tc.tile_pool(name="vext", bufs=2))
        ytp = ctx.enter_context(tc.tile_pool(name="yt", bufs=2))
        expp = ctx.enter_context(tc.tile_pool(name="expt", bufs=10))
        smallp = ctx.enter_context(tc.tile_pool(name="small", bufs=2))
        outp = ctx.enter_context(tc.tile_pool(name="outt", bufs=4))
        ps_acc = ctx.enter_context(tc.tile_pool(name="ps_acc", bufs=5, space="PSUM"))
        ps_sc = ctx.enter_context(tc.tile_pool(name="ps_sc", bufs=3, space="PSUM"))

        # [128, 64] tile holding I64 in partitions 0-63 and again in 64-127,
        # so each head's vT slice has an identity at its own base partition.
        ident = consts.tile([128, 64], F32)
        nc.gpsimd.memset(ident, 0.0)
        for half in range(2):
            nc.gpsimd.affine_select(
                out=ident[64 * half : 64 * half + 64, :],
                in_=ident[64 * half : 64 * half + 64, :],
                compare_op=mybir.AluOpType.not_equal,
                fill=1.0,
                base=0,
                pattern=[[-1, 64]],
                channel_multiplier=1,
            )
        ones_row = consts.tile([1, 64], F32R)
        nc.vector.memset(ones_row.bitcast(F32), 1.0)

        # weights: wc [1024, 384] -> [128, 8, 384] (kc chunks on free dim)
        w_sb = consts.tile([128, 8, 3 * DPC], F32R)
        nc.sync.dma_start(
            out=w_sb, in_=wc_d.ap().rearrange("(kc p) c -> p kc c", p=128)
        )
        wp_sb = consts.tile([128, C], F32R)
        nc.sync.dma_start(out=wp_sb, in_=wp_d.ap())
        bc_sb = consts.tile([128, 3], F32)
        nc.sync.dma_start(out=bc_sb, in_=bc_d.ap().rearrange("g p one -> p (g one)"))

        NKC = C // 128  # 8 contraction chunks for qkv
        NTC = T // TCH  # 4 token chunks per batch
        NQC = T // QCH  # 4 q-chunks per batch (per head)
        NVC = T // 128  # 16 v chunks per batch

        # -------- unit-based emission with explicit cross-phase interleave.
        # Each "unit" is a thunk emitting a small group of instructions.
        # Attention(b) is ACT-bound (exp), so qkv(b+1), vT(b+1) and proj(b-1)
        # units are spliced between its j-iterations to keep PE dense.
        state = {}

        def qkv_units(b):
            t0 = b * T
            st = state.setdefault(b, {})
            units = []

            def alloc(b=b, st=st):
                st["qT"] = qkvp.tile([128, T], F32R, name=f"qT_{b}", tag="qT")
                st["kT"] = qkvp.tile([128, T], F32R, name=f"kT_{b}", tag="kT")
                st["vT"] = qkvp.tile([128, T], F32, name=f"vT_{b}", tag="vT")
                st["xts"] = {}

            units.append(alloc)
            for tcb in range(NTC):

                def dma_u(tcb=tcb, st=st, t0=t0):
                    xts = []
                    for kc in range(NKC):
                        xt = xpool.tile(
                            [128, TCH], F32R, name=f"xt_{b}_{tcb}_{kc}", tag="xt"
                        )
                        nc.sync.dma_start(
                            out=xt,
                            in_=xT_d.ap()[
                                kc * 128 : (kc + 1) * 128,
                                t0 + tcb * TCH : t0 + (tcb + 1) * TCH,
                            ],
                        )
                        xts.append(xt)
                    st["xts"][tcb] = xts

                units.append(dma_u)
                for g in range(3):

                    def mm_u(tcb=tcb, g=g, st=st):
                        dest = [st["qT"], st["kT"], st["vT"]]
                        ps = ps_acc.tile(
                            [128, TCH], F32, name=f"qkvps_{b}_{tcb}_{g}", tag="acc"
                        )
                        for kc in range(NKC):
                            nc.tensor.matmul(
                                ps,
                                w_sb[:, kc, g * 128 : (g + 1) * 128],
                                st["xts"][tcb][kc],
                                start=(kc == 0),
                                stop=(kc == NKC - 1),
                            )
                        # psum -> sbuf with bias add, on DVE (ACT stays on exp)
                        nc.vector.tensor_scalar_add(
                            dest[g][:, tcb * TCH : (tcb + 1) * TCH],
                            ps,
                            bc_sb[:, g : g + 1],
                        )

                    units.append(mm_u)
            return units

        def vt_units(b):
            st = state.setdefault(b, {})
            units = []

            def alloc(st=st, b=b):
                st["vex"] = vexp.tile(
                    [128, HPC, NVC, 65], F32R, name=f"vex_{b}", tag="vex"
                )
                nc.vector.memset(st["vex"][:, :, :, 64:65].bitcast(F32), 1.0)

            units.append(alloc)
            for h in range(HPC):
                for j0 in range(0, NVC, 4):

                    def tr_u(h=h, j0=j0, st=st):
                        for j in range(j0, j0 + 4):
                            pt = ps_sc.tile(
                                [128, 64], F32, name=f"vtps_{b}_{h}_{j}", tag="sc"
                            )
                            nc.tensor.transpose(
                                pt,
                                st["vT"][64 * h : 64 * h + 64, j * 128 : (j + 1) * 128],
                                ident[64 * h : 64 * h + 64, :],
                            )
                            nc.vector.tensor_copy(st["vex"][:, h, j, 0:64], pt)

                    units.append(tr_u)
            return units

        def attn_units(b):
            st = state[b]
            units = []

            def alloc_yt(st=st, b=b):
                st["yT"] = ytp.tile([128, T], F32R, name=f"yT_{b}", tag="yT")

            units.append(alloc_yt)
            for h in range(HPC):

                def alloc_chains(h=h, st=st, b=b):
                    st[("yts", h)] = [
                        ps_acc.tile(
                            [65, QCH], F32, name=f"yt_{b}_{h}_{qc}", tag="acc"
                        )
                        for qc in range(NQC)
                    ]

                units.append(alloc_chains)
                for j in range(NVC):

                    def j_u(h=h, j=j, st=st, b=b):
                        qTh = st["qT"][64 * h : 64 * h + 64, :]
                        kTh = st["kT"][64 * h : 64 * h + 64, :]
                        k0 = j * KCH
                        exs = []
                        for qc in range(NQC):
                            q0 = qc * QCH
                            if k0 >= q0 + QCH:
                                continue
                            qlo = max(0, k0 - q0)
                            sc = ps_sc.tile(
                                [128, QCH], F32, name=f"sc_{b}_{h}_{j}_{qc}", tag="sc"
                            )
                            nc.tensor.matmul(
                                sc[:, qlo:QCH],
                                kTh[:, k0 : k0 + KCH],
                                qTh[:, q0 + qlo : q0 + QCH],
                                start=True,
                                stop=True,
                            )
                            ex = expp.tile(
                                [128, QCH], F32R, name=f"ex_{b}_{h}_{j}_{qc}", tag="ex"
                            )
                            nc.scalar.activation(
                                ex[:, qlo:QCH], sc[:, qlo:QCH], AF.Exp, scale=0.125
                            )
                            if k0 >= q0:
                                # diagonal 128-wide block: zero where k > q
                                nc.gpsimd.affine_select(
                                    out=ex[:, qlo : qlo + 128],
                                    in_=ex[:, qlo : qlo + 128],
                                    compare_op=mybir.AluOpType.is_ge,
                                    fill=0.0,
                                    base=0,
                                    pattern=[[1, 128]],
                                    channel_multiplier=-1,
                                )
                            exs.append((qc, ex, qlo))
                        for qc, ex, qlo in exs:
                            q0 = qc * QCH
                            nk = (q0 + QCH) // KCH
                            nc.tensor.matmul(
                                st[("yts", h)][qc][:, qlo:QCH],
                                st["vex"][:, h, j, :],
                                ex[:, qlo:QCH],
                                start=(j == 0),
                                stop=(j == nk - 1),
                            )

                    units.append(j_u)

                def norm_u(h=h, st=st, b=b):
                    # gather the 4 q-chunks' sums, then one Ln and one Exp
                    # over [1, 4*QCH]: 1/s = exp(-ln(s)) on ACT (same table
                    # set as Exp; AF.Reciprocal forces a ~1.3us table swap)
                    sums4 = smallp.tile(
                        [1, NQC * QCH], F32, name=f"sums_{b}_{h}", tag="lns"
                    )
                    for qc in range(NQC):
                        nc.vector.tensor_copy(
                            sums4[:, qc * QCH : (qc + 1) * QCH],
                            st[("yts", h)][qc][64:65, :],
                        )
                    recip4 = smallp.tile(
                        [1, NQC * QCH], F32R, name=f"rc_{b}_{h}", tag="recip"
                    )
                    nc.scalar.activation(
                        sums4, sums4, AF.Ln
                    )
                    nc.scalar.activation(recip4, sums4, AF.Exp, scale=-1.0)
                    for qc in range(NQC):
                        yt_ps = st[("yts", h)][qc]
                        q0 = qc * QCH
                        bc_ps = ps_sc.tile(
                            [64, QCH], F32, name=f"bc_{b}_{h}_{qc}", tag="sc"
                        )
                        nc.tensor.matmul(
                            bc_ps,
                            ones_row,
                            recip4[:, qc * QCH : (qc + 1) * QCH],
                            start=True,
                            stop=True,
                        )
                        bc_sb2 = smallp.tile(
                            [64, QCH], F32, name=f"bcs_{b}_{h}_{qc}", tag="bcast"
                        )
                        nc.vector.tensor_copy(bc_sb2, bc_ps)
                        nc.vector.tensor_mul(
                            st["yT"][64 * h : 64 * h + 64, q0 : q0 + QCH],
                            yt_ps[0:64, :],
                            bc_sb2,
                        )

                units.append(norm_u)
            return units

        def proj_units(b):
            st = state[b]
            t0 = b * T
            units = []
            for tcb in range(T // 128):

                def p_u(tcb=tcb, st=st, t0=t0, b=b):
                    for g in range(2):
                        ps = ps_acc.tile(
                            [128, 512], F32, name=f"pps_{b}_{tcb}_{g}", tag="acc"
                        )
                        nc.tensor.matmul(
                            ps,
                            st["yT"][:, tcb * 128 : (tcb + 1) * 128],
                            wp_sb[:, g * 512 : (g + 1) * 512],
                            start=True,
                            stop=True,
                        )
                        ot = outp.tile(
                            [128, 512], F32, name=f"ot_{b}_{tcb}_{g}", tag="ot"
                        )
                        nc.vector.tensor_copy(ot, ps)
                        nc.sync.dma_start(
                            out=out_d.ap()[
                                t0 + tcb * 128 : t0 + (tcb + 1) * 128,
                                g * 512 : (g + 1) * 512,
                            ],
                            in_=ot,
                        )

                units.append(p_u)
            return units

        def interleave(main, fill):
            """emit main units with fill units spread evenly between them"""
            out = []
            nf, nm = len(fill), len(main)
            fi = 0
            for mi, m in enumerate(main):
                out.append(m)
                want = (mi + 1) * nf // nm
                while fi < want:
                    out.append(fill[fi])
                    fi += 1
            out.extend(fill[fi:])
            return out

        for u in qkv_units(0) + vt_units(0):
            u()
        for b in range(B):
            main = attn_units(b)
            fill = []
            if b + 1 < B:
                fill += qkv_units(b + 1) + vt_units(b + 1)
            if b >= 1:
                fill += proj_units(b - 1)
            for u in interleave(main, fill):
                u()
        for u in proj_units(B - 1):
            u()

    _split_multi_waits(nc)
    return nc


_NC_CACHE = None


def _get_nc():
    global _NC_CACHE
    if _NC_CACHE is None:
        _NC_CACHE = build_kernel()
    return _NC_CACHE


def kernel_with_results(x, W_attn, b_attn, W_proj, b_proj, trace=False):
    x = np.asarray(x, dtype=np.float32)
    W_attn = np.asarray(W_attn, dtype=np.float32)
    b_attn = np.asarray(b_attn, dtype=np.float32)
    W_proj = np.asarray(W_proj, dtype=np.float32)
    b_proj = np.asarray(b_proj, dtype=np.float32)

    xT = np.ascontiguousarray(x.reshape(BT, C).T)  # [C, BT]
    in_maps = []
    for c in range(N_CORES):
        lo = c * DPC
        wc = np.ascontiguousarray(
            np.concatenate(
                [
                    W_attn[:, lo : lo + DPC],
                    W_attn[:, C + lo : C + lo + DPC],
                    W_attn[:, 2 * C + lo : 2 * C + lo + DPC],
                ],
                axis=1,
            )
        )
        bc = np.ascontiguousarray(
            np.stack(
                [
                    b_attn[lo : lo + DPC],
                    b_attn[C + lo : C + lo + DPC],
                    b_attn[2 * C + lo : 2 * C + lo + DPC],
                ]
            ).reshape(3, DPC, 1)
        )
        wp = np.ascontiguousarray(W_proj[lo : lo + DPC, :])
        in_maps.append({"xT": xT, "wc": wc, "bc": bc, "wp": wp})

    nc = _get_nc()
    res = run_bass_kernel_spmd(
        nc, in_maps, core_ids=list(range(N_CORES)), trace=trace
    )
    acc = np.zeros((BT, C), dtype=np.float64)
    for c in range(N_CORES):
        acc += res.results[c]["out"].astype(np.float64)
    out = (acc + b_proj.astype(np.float64)).astype(np.float32)
    return out.reshape(B, T, C), res


def kernel(x, W_attn, b_attn, W_proj, b_proj):
    out, _ = kernel_with_results(x, W_attn, b_attn, W_proj, b_proj)
    return out



# revision 6
# speedup vs baseline: 1.7432x; 1.7432x over previous
"""Causal self-attention (B=4, T=2048, C=1024, NH=16) on 8 TRN2 NeuronCores.

Sharding: tensor-parallel over heads - 2 heads per core. Each core computes
its slice of qkv, full causal attention for its heads, and a partial output
projection; the host sums the 8 partials and adds b_proj (plus an exact
host-side correction for the v-bias, which commutes through softmax).

v2 (from the 573us baseline, targeting the engine rooflines):
 - bf16 operands everywhere (inputs, weights, q/k/v, probs, y, outputs);
   PSUM accumulation stays fp32. Halves DMA (69.5 -> ~34 MB/core) and SBUF.
 - The 1/sqrt(HD)=1/8 score scale is folded into W_k on the host, so score
   PSUM values are already scaled and exp needs no scale immediate.
 - Scores for the head PAIR run concurrently on the PE via row tiling
   (K=64 each: tile_position (0,0) and (64,0)), writing adjacent PSUM banks.
 - One ACTIVATE computes exp for both heads ([128, 2, 512] strided read
   across two PSUM banks) - ACT's 352-cycle per-instruction overhead was
   ~40% of its busy time at [128,512] granularity.
 - v is computed directly in natural [token, dim] layout by swapping the
   matmul operands (x chunk stationary), eliminating the PE transposes.
 - Normalization is deferred: chains accumulate unnormalized y plus a
   denominator row (ones column in v); denominators for all 8 (head, qc)
   of a batch get one batched Ln+Exp reciprocal on 8 ACT lanes, then a
   K=8 select-matmul broadcasts 1/den across 64 partitions for the DVE mul.
 - Attention is software-pipelined (p@v lags scores by 2 steps; score PSUM
   double-buffered) so the PE never waits on ACT; qkv(b+1) and proj(b-1)
   units are interleaved between attention steps to keep the PE dense and
   the HAM clock-gate warm (the baseline spent its second half at 1.2 GHz).
"""

import sys

import numpy as np

try:
    import concourse.bass as bass
except ImportError:  # grading container may not have it on sys.path
    sys.path.insert(0, "/opt/trn_rl_repo")
    import concourse.bass as bass

from contextlib import ExitStack

import ml_dtypes
import concourse.mybir as mybir
import concourse.tile as tile
from concourse.bass_utils import run_bass_kernel_spmd


B, T, C, NH, HD = 4, 2048, 1024, 16, 64
N_CORES = 8
HPC = NH // N_CORES  # heads per core = 2
DPC = HPC * HD  # dims per core = 128
BT = B * T  # 8192
QCH = 512  # q-chunk
TCH = 512  # token chunk for qkv
NKC = C // 128  # 8 contraction chunks for qkv
NTC = T // TCH  # 4 token chunks per batch
NQC = T // QCH  # 4 q-chunks per batch (per head)
NJ = T // 128  # 16 key chunks per batch
F32 = mybir.dt.float32
BF16 = mybir.dt.bfloat16
AF = mybir.ActivationFunctionType
BF = ml_dtypes.bfloat16

MEGA_EXP = True  # one ACTIVATE across both heads' score banks


def _split_multi_waits(nc):
    """Walrus in this container accepts only ONE sync wait per instruction.
    Hoist extra waits onto same-engine NoOps inserted just before."""
    n = 0
    for f in nc.m.functions:
        for b in f.blocks:
            insts = b.instructions
            if not any(
                i.sync_info is not None
                and i.sync_info.on_wait
                and len(i.sync_info.on_wait) > 1
                for i in insts
            ):
                continue
            new = []
            for ins in insts:
                si = ins.sync_info
                if si is not None and si.on_wait and len(si.on_wait) > 1:
                    waits = list(si.on_wait)
                    for w in waits[:-1]:
                        nop = mybir.InstNoOp(
                            name=f"{ins.name}-ws{n}", ins=[], outs=[]
                        )
                        nop.engine = ins.engine
                        nop.bass_nofuse = True
                        nop.sync_info = mybir.SyncInfo(on_wait=[w], on_update=[])
                        if ins.debug is not None:
                            nop.debug = ins.debug
                        new.append(nop)
                        n += 1
                    ins.sync_info = mybir.SyncInfo(
                        on_wait=[waits[-1]], on_update=list(si.on_update or [])
                    )
                new.append(ins)
            b.instructions = new
    return n


def build_kernel():
    nc = bass.Bass("TRN2", target_bir_lowering=False, debug=False, num_devices=N_CORES)
    xT_d = nc.dram_tensor("xT", [C, BT], BF16, kind="ExternalInput")
    wc_d = nc.dram_tensor("wc", [C, 3 * DPC], BF16, kind="ExternalInput")
    bc_d = nc.dram_tensor("bc", [2, DPC, 1], F32, kind="ExternalInput")
    wp_d = nc.dram_tensor("wp", [DPC, C], BF16, kind="ExternalInput")
    out_d = nc.dram_tensor("out", [BT, C], BF16, kind="ExternalOutput")

    with tile.TileContext(nc) as tc, ExitStack() as ctx:
        consts = ctx.enter_context(tc.tile_pool(name="consts", bufs=1))
        xpool = ctx.enter_context(tc.tile_pool(name="x", bufs=16))
        qkvp = ctx.enter_context(tc.tile_pool(name="qkv", bufs=2))
        vexp = ctx.enter_context(tc.tile_pool(name="vext", bufs=2))
        ytup = ctx.enter_context(tc.tile_pool(name="ytu", bufs=2))
        ytp = ctx.enter_context(tc.tile_pool(name="yt", bufs=2))
        expp = ctx.enter_context(tc.tile_pool(name="expt", bufs=4))
        smallp = ctx.enter_context(tc.tile_pool(name="small", bufs=2))
        outp = ctx.enter_context(tc.tile_pool(name="outt", bufs=4))
        # PSUM: scp 2 bufs x [128,2,512]f32 (4 banks) + chp 2 x [65,512]
        # (2 banks) + acc 2 x [128,512] (2 banks) = exactly 8 banks.
        scp = ctx.enter_context(tc.tile_pool(name="ps_sc", bufs=2, space="PSUM"))
        chp = ctx.enter_context(tc.tile_pool(name="ps_ch", bufs=2, space="PSUM"))
        accp = ctx.enter_context(tc.tile_pool(name="ps_acc", bufs=2, space="PSUM"))

        # weights: wc [1024, 384] -> [128, 8, 384] (kc chunks on free dim)
        w_sb = consts.tile([128, NKC, 3 * DPC], BF16)
        nc.sync.dma_start(
            out=w_sb, in_=wc_d.ap().rearrange("(kc p) c -> p kc c", p=128)
        )
        wp_sb = consts.tile([128, C], BF16)
        nc.sync.dma_start(out=wp_sb, in_=wp_d.ap())
        bc_sb = consts.tile([128, 2], F32)
        nc.sync.dma_start(out=bc_sb, in_=bc_d.ap().rearrange("g p one -> p (g one)"))
        # sel4[32*qc, qc*64:(qc+1)*64] = 1: select-matmul broadcasts recip row
        # 32*qc across 64 output partitions (partition bases must be
        # 32-aligned on TRN2, hence rows {0,32,64,96}).
        sel4 = consts.tile([97, NQC * 64], BF16)
        nc.vector.memset(sel4, 0.0)
        for qc in range(NQC):
            nc.vector.memset(sel4[32 * qc : 32 * qc + 1, qc * 64 : (qc + 1) * 64], 1.0)

        state = {}

        # ---------------- qkv units (per batch) ----------------
        def qkv_units(b):
            t0 = b * T
            st = state.setdefault(b, {})
            units = []

            def alloc(b=b, st=st):
                st["qT"] = qkvp.tile([128, T], BF16, name=f"qT_{b}", tag="qT")
                st["kT"] = qkvp.tile([128, T], BF16, name=f"kT_{b}", tag="kT")
                st["vex"] = vexp.tile(
                    [128, NJ, HPC, 65], BF16, name=f"vex_{b}", tag="vex"
                )
                nc.vector.memset(st["vex"][:, :, :, 64:65], 1.0)
                st["yTu"] = ytup.tile([128, T], BF16, name=f"yTu_{b}", tag="yTu")
                # per-head denominator tiles; rows {0,32,64,96} = qc 0..3.
                # memset 1.0 so unused rows stay finite through Ln/Exp.
                st["den"] = [
                    smallp.tile([97, 512], F32, name=f"den_{b}_{h}", tag="den")
                    for h in range(HPC)
                ]
                st["recip"] = [
                    smallp.tile([97, 512], BF16, name=f"recip_{b}_{h}", tag="recip")
                    for h in range(HPC)
                ]
                for h in range(HPC):
                    nc.vector.memset(st["den"][h], 1.0)
                st["xts"] = {}

            units.append(alloc)
            for tcb in range(NTC):

                def dma_u(tcb=tcb, st=st, t0=t0, b=b):
                    xts = []
                    for kc in range(NKC):
                        xt = xpool.tile(
                            [128, TCH], BF16, name=f"xt_{b}_{tcb}_{kc}", tag="xt"
                        )
                        nc.sync.dma_start(
                            out=xt,
                            in_=xT_d.ap()[
                                kc * 128 : (kc + 1) * 128,
                                t0 + tcb * TCH : t0 + (tcb + 1) * TCH,
                            ],
                        )
                        xts.append(xt)
                    st["xts"][tcb] = xts

                units.append(dma_u)
                for g in range(2):  # 0 = q, 1 = k

                    def qk_u(tcb=tcb, g=g, st=st, b=b):
                        dest = [st["qT"], st["kT"]][g]
                        ps = accp.tile(
                            [128, TCH], F32, name=f"qkps_{b}_{tcb}_{g}", tag="acc"
                        )
                        for kc in range(NKC):
                            nc.tensor.matmul(
                                ps,
                                w_sb[:, kc, g * 128 : (g + 1) * 128],
                                st["xts"][tcb][kc],
                                start=(kc == 0),
                                stop=(kc == NKC - 1),
                            )
                        nc.vector.tensor_scalar_add(
                            dest[:, tcb * TCH : (tcb + 1) * TCH],
                            ps,
                            bc_sb[:, g : g + 1],
                        )

                    units.append(qk_u)

                for shalf in range(2):  # v natural: 2 units of 2 token-subchunks

                    def v_u(tcb=tcb, shalf=shalf, st=st, b=b):
                        vps = accp.tile(
                            [128, 2, HPC, 64],
                            F32,
                            name=f"vps_{b}_{tcb}_{shalf}",
                            tag="acc",
                        )
                        for si in range(2):
                            s = shalf * 2 + si
                            for kc in range(NKC):
                                nc.tensor.matmul(
                                    vps[:, si],
                                    st["xts"][tcb][kc][:, s * 128 : (s + 1) * 128],
                                    w_sb[:, kc, 2 * DPC : 3 * DPC],
                                    start=(kc == 0),
                                    stop=(kc == NKC - 1),
                                )
                        for si in range(2):
                            s = shalf * 2 + si
                            j = tcb * 4 + s
                            nc.vector.tensor_copy(
                                st["vex"][:, j, :, 0:64], vps[:, si]
                            )

                    units.append(v_u)
            return units

        # ---------------- attention units (per batch) ----------------
        def attn_units(b):
            st = state[b]
            units = []

            def alloc_yt(st=st, b=b):
                st["yT"] = ytp.tile([128, T], BF16, name=f"yT_{b}", tag="yT")

            units.append(alloc_yt)
            for qc in range(NQC):
                nj = 4 * qc + 4  # j in [0, nj)
                q0 = qc * QCH

                def alloc_ch(qc=qc, st=st, b=b):
                    st[("ch", qc)] = [
                        chp.tile([65, QCH], F32, name=f"ch_{b}_{qc}_{h}", tag="ch")
                        for h in range(HPC)
                    ]

                units.append(alloc_ch)

                # software pipeline: emit scores(j)+exp(j), then pv(j-2)
                def sc_u(j, qc=qc, q0=q0, st=st, b=b):
                    qlo = max(0, j * 128 - q0)
                    sc2 = scp.tile(
                        [128, HPC, QCH], F32, name=f"sc_{b}_{qc}_{j}", tag="sc"
                    )
                    ex2 = expp.tile(
                        [128, HPC, QCH], BF16, name=f"ex_{b}_{qc}_{j}", tag="ex"
                    )
                    for h in range(HPC):
                        nc.tensor.matmul(
                            sc2[:, h, qlo:QCH],
                            st["kT"][64 * h : 64 * h + 64, j * 128 : (j + 1) * 128],
                            st["qT"][64 * h : 64 * h + 64, q0 + qlo : q0 + QCH],
                            start=True,
                            stop=True,
                            tile_position=(64 * h, 0),
                        )
                    if MEGA_EXP:
                        nc.scalar.activation(
                            ex2[:, :, qlo:QCH], sc2[:, :, qlo:QCH], AF.Exp
                        )
                    else:
                        for h in range(HPC):
                            nc.scalar.activation(
                                ex2[:, h, qlo:QCH], sc2[:, h, qlo:QCH], AF.Exp
                            )
                    if j * 128 >= q0:  # diagonal block: zero where k > q
                        for h in range(HPC):
                            nc.gpsimd.affine_select(
                                out=ex2[:, h, qlo : qlo + 128],
                                in_=ex2[:, h, qlo : qlo + 128],
                                compare_op=mybir.AluOpType.is_ge,
                                fill=0.0,
                                base=0,
                                pattern=[[1, 128]],
                                channel_multiplier=-1,
                            )
                    st[("ex", qc, j)] = ex2

                def pv_u(j, qc=qc, q0=q0, nj=nj, st=st, b=b):
                    qlo = max(0, j * 128 - q0)
                    ex2 = st.pop(("ex", qc, j))
                    for h in range(HPC):
                        nc.tensor.matmul(
                            st[("ch", qc)][h][:, qlo:QCH],
                            st["vex"][:, j, h, :],
                            ex2[:, h, qlo:QCH],
                            start=(j == 0),
                            stop=(j == nj - 1),
                        )

                for j in range(nj):
                    units.append(lambda j=j, f=sc_u: f(j))
                    if j >= 2:
                        units.append(lambda j=j, f=pv_u: f(j - 2))
                units.append(lambda nj=nj, f=pv_u: f(nj - 2))
                units.append(lambda nj=nj, f=pv_u: f(nj - 1))

                def qc_end(qc=qc, q0=q0, st=st, b=b):
                    for h in range(HPC):
                        ch = st[("ch", qc)][h]
                        nc.vector.tensor_copy(
                            st["yTu"][64 * h : 64 * h + 64, q0 : q0 + QCH],
                            ch[0:64, :],
                        )
                        nc.vector.tensor_copy(
                            st["den"][h][32 * qc : 32 * qc + 1, :], ch[64:65, :]
                        )
                    del st[("ch", qc)]

                units.append(qc_end)

            def recip_u(st=st, b=b):
                # 1/den = exp(-ln(den)), batched over the 4 qc rows per head
                for h in range(HPC):
                    nc.scalar.activation(st["den"][h], st["den"][h], AF.Ln)
                    nc.scalar.activation(
                        st["recip"][h], st["den"][h], AF.Exp, scale=-1.0
                    )

            units.append(recip_u)
            for h in range(HPC):

                def norm_u(h=h, st=st, b=b):
                    for qc in range(NQC):
                        q0 = qc * QCH
                        bc_ps = accp.tile(
                            [128, QCH], F32, name=f"bc_{b}_{h}_{qc}", tag="acc"
                        )
                        nc.tensor.matmul(
                            bc_ps[0:64, :],
                            sel4[:, qc * 64 : (qc + 1) * 64],
                            st["recip"][h],
                            start=True,
                            stop=True,
                        )
                        nc.vector.tensor_tensor(
                            out=st["yT"][64 * h : 64 * h + 64, q0 : q0 + QCH],
                            in0=st["yTu"][64 * h : 64 * h + 64, q0 : q0 + QCH],
                            in1=bc_ps[0:64, :],
                            op=mybir.AluOpType.mult,
                        )

                units.append(norm_u)
            return units

        # ---------------- projection units (per batch) ----------------
        def proj_units(b):
            st = state[b]
            t0 = b * T
            units = []
            for tcb in range(T // 128):

                def p_u(tcb=tcb, st=st, t0=t0, b=b):
                    for g in range(2):
                        ps = accp.tile(
                            [128, 512], F32, name=f"pps_{b}_{tcb}_{g}", tag="acc"
                        )
                        nc.tensor.matmul(
                            ps,
                            st["yT"][:, tcb * 128 : (tcb + 1) * 128],
                            wp_sb[:, g * 512 : (g + 1) * 512],
                            start=True,
                            stop=True,
                        )
                        ot = outp.tile(
                            [128, 512], BF16, name=f"ot_{b}_{tcb}_{g}", tag="ot"
                        )
                        nc.vector.tensor_copy(ot, ps)
                        nc.sync.dma_start(
                            out=out_d.ap()[
                                t0 + tcb * 128 : t0 + (tcb + 1) * 128,
                                g * 512 : (g + 1) * 512,
                            ],
                            in_=ot,
                        )

                units.append(p_u)
            return units

        def interleave(main, fill):
            """emit main units with fill units spread evenly between them"""
            out = []
            nf, nm = len(fill), len(main)
            fi = 0
            for mi, m in enumerate(main):
                out.append(m)
                want = (mi + 1) * nf // nm
                while fi < want:
                    out.append(fill[fi])
                    fi += 1
            out.extend(fill[fi:])
            return out

        for u in qkv_units(0):
            u()
        for b in range(B):
            main = attn_units(b)
            fill = []
            if b + 1 < B:
                fill += qkv_units(b + 1)
            if b >= 1:
                fill += proj_units(b - 1)
            for u in interleave(main, fill):
                u()
        for u in proj_units(B - 1):
            u()

    _split_multi_waits(nc)
    return nc


_NC_CACHE = None


def _get_nc():
    global _NC_CACHE
    if _NC_CACHE is None:
        _NC_CACHE = build_kernel()
    return _NC_CACHE


def kernel_with_results(x, W_attn, b_attn, W_proj, b_proj, trace=False):
    x = np.asarray(x, dtype=np.float32)
    W_attn = np.asarray(W_attn, dtype=np.float32)
    b_attn = np.asarray(b_attn, dtype=np.float32)
    W_proj = np.asarray(W_proj, dtype=np.float32)
    b_proj = np.asarray(b_proj, dtype=np.float32)

    xT = np.ascontiguousarray(x.reshape(BT, C).T).astype(BF)  # [C, BT] bf16
    in_maps = []
    for c in range(N_CORES):
        lo = c * DPC
        wc = np.ascontiguousarray(
            np.concatenate(
                [
                    W_attn[:, lo : lo + DPC],
                    W_attn[:, C + lo : C + lo + DPC] * 0.125,  # fold 1/sqrt(HD)
                    W_attn[:, 2 * C + lo : 2 * C + lo + DPC],
                ],
                axis=1,
            )
        ).astype(BF)
        bc = np.ascontiguousarray(
            np.stack(
                [
                    b_attn[lo : lo + DPC],
                    b_attn[C + lo : C + lo + DPC] * 0.125,
                ]
            ).reshape(2, DPC, 1)
        ).astype(np.float32)
        wp = np.ascontiguousarray(W_proj[lo : lo + DPC, :]).astype(BF)
        in_maps.append({"xT": xT, "wc": wc, "bc": bc, "wp": wp})

    nc = _get_nc()
    res = run_bass_kernel_spmd(
        nc, in_maps, core_ids=list(range(N_CORES)), trace=trace
    )
    acc = np.zeros((BT, C), dtype=np.float64)
    for c in range(N_CORES):
        acc += np.asarray(res.results[c]["out"]).astype(np.float64)
    # v-bias commutes through softmax: y += b_v, so out += b_v @ W_proj
    vshift = b_attn[2 * C : 3 * C].astype(np.float64) @ W_proj.astype(np.float64)
    out = (acc + vshift + b_proj.astype(np.float64)).astype(np.float32)
    return out.reshape(B, T, C), res


def kernel(x, W_attn, b_attn, W_proj, b_proj):
    out, _ = kernel_with_results(x, W_attn, b_attn, W_proj, b_proj)
    return out
